# revision 9
# baseline (speedup 1.0000x reference)
"""Trainium2 Bass kernel for nn_Basenet_collective (ragged RoI-box MLP head).

Computes, for X=[80,13,26400] box features with ragged per-frame validity:
    h = relu(X @ w_emb + b_emb)                      [80,13,1024]
    actions = (h @ w_act + b_act) * valid_mask       [80,13,6]
    activities = max_pool_valid(h) @ w_acty + b_acty [80,5]

Distribution over 8 NeuronCores (one trn2 chip):
  - Host compacts the ragged box axis (only sum(bboxes_num) of the 80*13
    box slots contribute to the output), groups frames by box count, and
    transposes X to [K2D, V].
  - The 26400-deep contraction is split 8 ways (3300, padded to 26x128,
    per core); each core computes a partial H^T = w^T @ x of shape
    [1024, V] on the tensor engine (bf16 in, fp32 PSUM accumulation).
    Host packs each core's x and w k-tiles side by side as [128, KT*cols]
    so DMA rows are tens of KB and stream at full HBM rate.
  - Box columns are processed in a couple of chunks (>=260 cols each so
    LDWEIGHTS hides behind the matmul stream); each chunk's partial is
    ReduceScatter(add)-ed across cores as soon as it is ready, so the
    collectives overlap the remaining matmul work. After the last RS,
    core i holds feature rows [128i:128(i+1)] of the summed H^T.
  - Stage 2 (bias+relu, action scores, grouped max-pool, activity scores)
    is feature-sharded and frame-local on the device.
  - Host sums the 8 feature-shard partials of the two small outputs,
    adds biases, and scatters back to the original ragged layout.
"""

import numpy as np

_BT = 80
_MAXN = 13
_K2D = 26400
_NFB = 1024
_A = 6
_G = 5
_NC = 8
_KC = _K2D // _NC   # 3300 contraction rows per core
_KT = -(-_KC // 128)  # 26 k-tiles (last padded with zeros to 128)
_FC = _NFB // _NC   # 128 feature rows per core after ReduceScatter
_KG = 5             # k-tiles per load-group DMA

_MM_DTYPE = "bf16"  # 'bf16' | 'f32' — stage-1 matmul input dtype
_CC_DTYPE = "f16"   # 'f16' | 'f32' — ReduceScatter payload dtype


def _chunk_cols(Vp):
    """Split Vp box columns into matmul/RS chunks: each <=512, mult of 16,
    and >=260 where possible (so per-matmul LDWEIGHTS stays hidden)."""
    n = -(-Vp // 512)
    if n == 1 and Vp >= 544:
        n = 2
    base = (-(-Vp // n) + 15) // 16 * 16
    chunks = []
    off = 0
    while off < Vp:
        sz = min(base, Vp - off)
        chunks.append((off, sz))
        off += sz
    assert all(s <= 512 for _, s in chunks)
    return chunks


def _plan(bboxes_num):
    """Host-side plan: compaction order, pooling groups, column chunks."""
    n = np.asarray(bboxes_num).astype(np.int64)
    assert n.shape == (_BT,) and n.min() >= 1 and n.max() <= _MAXN
    order = np.argsort(n, kind="stable")          # frames sorted by box count
    ns = n[order]
    V = int(n.sum())
    Vp = ((max(V, 64) + 15) // 16) * 16           # padded compacted box count

    # flat indices into the [80*13] box axis, frames in sorted order
    flat_idx = np.concatenate(
        [np.arange(t * _MAXN, t * _MAXN + int(n[t])) for t in order]
    )

    # pooling groups: runs of frames with equal box count n -> one strided
    # [128, cnt, n] max-reduce each. (frame_off, cnt, nval, col_off)
    groups = []
    col = 0
    f = 0
    for val in np.unique(ns):
        cnt = int((ns == val).sum())
        groups.append((f, cnt, int(val), col))
        f += cnt
        col += cnt * int(val)
    assert col == V and f == _BT

    return n, order, flat_idx, V, Vp, groups, _chunk_cols(Vp)


def _build(Vp, groups, chunks):
    """Build the SPMD bass program (identical on all 8 cores)."""
    import concourse.bass as bass
    import concourse.tile as tile
    from concourse import bacc, mybir

    f32 = mybir.dt.float32
    in_dt = mybir.dt.bfloat16 if _MM_DTYPE == "bf16" else f32
    cc_dt = mybir.dt.float16 if _CC_DTYPE == "f16" else f32

    nc = bacc.Bacc(
        "TRN2",
        target_bir_lowering=False,
        debug=False,
        enable_asserts=True,
        num_devices=_NC,
    )

    # packed layouts: row p, column k*width+c  <->  logical [k*128+p, c]
    x_d = nc.dram_tensor("x", [128, _KT * Vp], in_dt, kind="ExternalInput")
    w_d = nc.dram_tensor("w", [128, _KT * _NFB], in_dt, kind="ExternalInput")
    be_d = nc.dram_tensor("be", [_FC, 1], f32, kind="ExternalInput")
    wa_d = nc.dram_tensor("wa", [_FC, _A], f32, kind="ExternalInput")
    wy_d = nc.dram_tensor("wy", [_FC, _G], f32, kind="ExternalInput")
    oa_d = nc.dram_tensor("out_act", [_A, Vp], f32, kind="ExternalOutput")
    oy_d = nc.dram_tensor("out_acty", [_G, _BT], f32, kind="ExternalOutput")
    # per-chunk collective bounce buffers (internal DRAM)
    hp_d = [
        nc.dram_tensor(f"hpart{ci}", [_NFB, nsz], cc_dt)
        for ci, (_, nsz) in enumerate(chunks)
    ]
    rs_d = [
        nc.dram_tensor(f"rsout{ci}", [_FC, nsz], cc_dt)
        for ci, (_, nsz) in enumerate(chunks)
    ]

    kgroups = [(g, min(_KG, _KT - g)) for g in range(0, _KT, _KG)]

    with tile.TileContext(nc) as tc:
        with (
            tc.tile_pool(name="sb", bufs=1) as sb,
            tc.tile_pool(name="psum", bufs=1, space="PSUM") as psum,
        ):
            # load x and w in big contiguous group-DMAs; slice k-tiles in SBUF
            w_g = {}
            x_g = {}
            for g0, gn in kgroups:
                wg = sb.tile([128, gn * _NFB], in_dt, tag=f"w{g0}", bufs=1,
                             name=f"wg{g0}")
                nc.scalar.dma_start(wg[:], w_d[:, g0 * _NFB : (g0 + gn) * _NFB])
                xg = sb.tile([128, gn * Vp], in_dt, tag=f"x{g0}", bufs=1,
                             name=f"xg{g0}")
                nc.sync.dma_start(xg[:], x_d[:, g0 * Vp : (g0 + gn) * Vp])
                for j in range(gn):
                    w_g[g0 + j] = wg[:, j * _NFB : (j + 1) * _NFB]
                    x_g[g0 + j] = xg[:, j * Vp : (j + 1) * Vp]

            # stage 1: per column chunk, H^T partial = sum_k w[k]^T @ x[k],
            # k-outer m-inner over 8 persistent PSUM banks so the PE paces
            # just above the DMA stream and stays HAM-warm.
            for ci, (co, nsz) in enumerate(chunks):
                ps = [
                    psum.tile([128, nsz], f32, tag="ps", bufs=8,
                              name=f"ps{ci}_{m}")
                    for m in range(8)
                ]
                for ki in range(_KT):
                    for m in range(8):
                        nc.tensor.matmul(
                            ps[m][:],
                            w_g[ki][:, m * 128 : (m + 1) * 128],
                            x_g[ki][:, co : co + nsz],
                            start=(ki == 0),
                            stop=(ki == _KT - 1),
                        )
                for m in range(8):
                    st = sb.tile([128, nsz], cc_dt, tag="st", bufs=6,
                                 name=f"st{ci}_{m}")
                    nc.vector.tensor_copy(st[:], ps[m][:])
                    nc.gpsimd.dma_start(
                        hp_d[ci][m * 128 : (m + 1) * 128, :], st[:]
                    )
                # chunk's partial is complete: reduce+scatter it across cores
                # while later chunks keep the PE busy
                nc.gpsimd.collective_compute(
                    "ReduceScatter",
                    mybir.AluOpType.add,
                    replica_groups=[list(range(_NC))],
                    ins=[hp_d[ci][:]],
                    outs=[rs_d[ci][:]],
                )

            # stage 2 (feature-sharded): bias+relu per chunk as its RS lands
            bt = sb.tile([_FC, 1], f32, tag="bt", bufs=1)
            nc.sync.dma_start(bt[:], be_d[:])
            wa = sb.tile([_FC, _A], f32, tag="wa", bufs=1)
            nc.sync.dma_start(wa[:], wa_d[:])
            wy = sb.tile([_FC, _G], f32, tag="wy", bufs=1)
            nc.sync.dma_start(wy[:], wy_d[:])

            hr = sb.tile([_FC, Vp], f32, tag="hr", bufs=1)
            oa_sb = sb.tile([_A, Vp], f32, tag="oasb", bufs=1)
            for ci, (co, nsz) in enumerate(chunks):
                h2 = sb.tile([_FC, nsz], cc_dt, tag="h2", bufs=2, name=f"h2_{ci}")
                nc.sync.dma_start(h2[:], rs_d[ci][:])
                nc.scalar.activation(
                    hr[:, co : co + nsz],
                    h2[:],
                    mybir.ActivationFunctionType.Relu,
                    bias=bt[:, 0:1],
                )
                pa = psum.tile([_A, nsz], f32, tag="ps", bufs=8, name=f"pa{ci}")
                nc.tensor.matmul(
                    pa[:], wa[:], hr[:, co : co + nsz], start=True, stop=True
                )
                nc.vector.tensor_copy(oa_sb[:, co : co + nsz], pa[:])
            nc.sync.dma_start(oa_d[:], oa_sb[:])

            # masked max-pool: one strided reduce per group of equal box count
            pooled = sb.tile([_FC, _BT], f32, tag="pl", bufs=1)
            for fo, cnt, nv, co in groups:
                src = hr[:, co : co + cnt * nv].rearrange("p (c n) -> p c n", n=nv)
                nc.vector.reduce_max(
                    pooled[:, fo : fo + cnt], src, axis=mybir.AxisListType.X
                )

            py = psum.tile([_G, _BT], f32, tag="ps", bufs=8, name="py")
            nc.tensor.matmul(py[:], wy[:], pooled[:], start=True, stop=True)
            oy_sb = sb.tile([_G, _BT], f32, tag="oysb", bufs=1)
            nc.vector.tensor_copy(oy_sb[:], py[:])
            nc.sync.dma_start(oy_d[:], oy_sb[:])

    nc.compile()
    return nc


def _pack_ktiles(a, width):
    """[KT*128(padded), width] -> [128, KT*width] with k-tiles side by side."""
    KT = a.shape[0] // 128
    return np.ascontiguousarray(
        a.reshape(KT, 128, width).transpose(1, 0, 2).reshape(128, KT * width)
    )


def _run(inputs, trace=False, trace_kwargs=None):
    """Shard, run on 8 cores, gather. Returns (actions, activities, results)."""
    from concourse.bass_utils import run_bass_kernel_spmd

    boxes_features_flat = np.asarray(inputs["boxes_features_flat"], np.float32)
    w_emb = np.asarray(inputs["w_emb"], np.float32)
    b_emb = np.asarray(inputs["b_emb"], np.float32)
    w_act = np.asarray(inputs["w_act"], np.float32)
    b_act = np.asarray(inputs["b_act"], np.float32)
    w_acty = np.asarray(inputs["w_acty"], np.float32)
    b_acty = np.asarray(inputs["b_acty"], np.float32)
    bboxes_num = np.asarray(inputs["bboxes_num"])

    n, order, flat_idx, V, Vp, groups, chunks = _plan(bboxes_num)

    # host marshalling: compact + transpose X to [K2D, Vp], cast, and pack
    X = boxes_features_flat.reshape(_BT * _MAXN, _K2D)
    XT = np.zeros((_K2D, Vp), np.float32)
    XT[:, :V] = X[flat_idx].T

    if _MM_DTYPE == "bf16":
        import ml_dtypes

        dt_in = ml_dtypes.bfloat16
    else:
        dt_in = np.float32
    XT = XT.astype(dt_in)
    w_in = w_emb.astype(dt_in)

    kc_pad = _KT * 128  # 3328
    in_maps = []
    for i in range(_NC):
        xi = np.zeros((kc_pad, Vp), dt_in)
        xi[:_KC] = XT[_KC * i : _KC * (i + 1)]
        wi = np.zeros((kc_pad, _NFB), dt_in)
        wi[:_KC] = w_in[_KC * i : _KC * (i + 1)]
        in_maps.append(
            {
                "x": _pack_ktiles(xi, Vp),
                "w": _pack_ktiles(wi, _NFB),
                "be": np.ascontiguousarray(
                    b_emb[_FC * i : _FC * (i + 1)].reshape(_FC, 1)
                ),
                "wa": np.ascontiguousarray(w_act[_FC * i : _FC * (i + 1)]),
                "wy": np.ascontiguousarray(w_acty[_FC * i : _FC * (i + 1)]),
            }
        )

    nc = _build(Vp, groups, chunks)
    res = run_bass_kernel_spmd(
        nc,
        in_maps,
        list(range(_NC)),
        trace=trace,
        **(trace_kwargs or {}),
    )

    # gather: sum feature-shard partials, add biases, scatter to ragged layout
    act_T = np.zeros((_A, Vp), np.float32)
    acty_T = np.zeros((_G, _BT), np.float32)
    for i in range(_NC):
        act_T += res.results[i]["out_act"]
        acty_T += res.results[i]["out_acty"]

    actions = np.zeros((_BT * _MAXN, _A), np.float32)
    actions[flat_idx] = act_T[:, :V].T + b_act[None, :]
    actions = actions.reshape(_BT, _MAXN, _A)

    activities = np.zeros((_BT, _G), np.float32)
    activities[order] = acty_T.T + b_acty[None, :]

    return actions, activities, res


def kernel(**inputs):
    actions, activities, _ = _run(inputs, trace=False)
    return actions, activities


# revision 11
# speedup vs baseline: 1.1199x; 1.1199x over previous
"""Trainium2 Bass kernel for nn_Basenet_collective (ragged RoI-box MLP head).

Computes, for X=[80,13,26400] box features with ragged per-frame validity:
    h = relu(X @ w_emb + b_emb)                      [80,13,1024]
    actions = (h @ w_act + b_act) * valid_mask       [80,13,6]
    activities = max_pool_valid(h) @ w_acty + b_acty [80,5]

Distribution over 8 NeuronCores (one trn2 chip):
  - Host compacts the ragged box axis (only sum(bboxes_num) of the 80*13
    box slots contribute to the output), groups frames by box count, and
    transposes X to [K2D, V].
  - The 26400-deep contraction is split 8 ways (3300, padded to 26x128,
    per core); each core computes a partial H^T = w^T @ x of shape
    [1024, V] on the tensor engine (bf16 in, fp32 PSUM accumulation).
    Host packs each core's x and w k-tiles side by side as [128, KT*cols]
    so DMA rows are tens of KB and stream at full HBM rate.
  - Box columns are processed in a couple of chunks (>=260 cols each so
    LDWEIGHTS hides behind the matmul stream); each chunk's partial is
    ReduceScatter(add)-ed across cores as soon as it is ready, so the
    collectives overlap the remaining matmul work. After the last RS,
    core i holds feature rows [128i:128(i+1)] of the summed H^T.
  - Stage 2 (bias+relu, action scores, grouped max-pool, activity scores)
    is feature-sharded and frame-local on the device.
  - Host sums the 8 feature-shard partials of the two small outputs,
    adds biases, and scatters back to the original ragged layout.
"""

import numpy as np

_BT = 80
_MAXN = 13
_K2D = 26400
_NFB = 1024
_A = 6
_G = 5
_NC = 8
_KC = _K2D // _NC   # 3300 contraction rows per core
_KT = -(-_KC // 128)  # 26 k-tiles (last padded with zeros to 128)
_FC = _NFB // _NC   # 128 feature rows per core after ReduceScatter
_KG = 5             # k-tiles per load-group DMA

_MM_DTYPE = "bf16"  # 'bf16' | 'f32' — stage-1 matmul input dtype
_CC_DTYPE = "f16"   # 'f16' | 'f32' — ReduceScatter payload dtype


def _chunk_cols(Vp):
    """Split Vp box columns into matmul/RS chunks: each <=512, mult of 16,
    and >=260 where possible (so per-matmul LDWEIGHTS stays hidden)."""
    n = -(-Vp // 512)
    if n == 1 and Vp >= 544:
        n = 2
    base = (-(-Vp // n) + 15) // 16 * 16
    chunks = []
    off = 0
    while off < Vp:
        sz = min(base, Vp - off)
        chunks.append((off, sz))
        off += sz
    assert all(s <= 512 for _, s in chunks)
    return chunks


def _plan(bboxes_num):
    """Host-side plan: compaction order, pooling groups, column chunks."""
    n = np.asarray(bboxes_num).astype(np.int64)
    assert n.shape == (_BT,) and n.min() >= 1 and n.max() <= _MAXN
    order = np.argsort(n, kind="stable")          # frames sorted by box count
    ns = n[order]
    V = int(n.sum())
    Vp = ((max(V, 64) + 15) // 16) * 16           # padded compacted box count

    # flat indices into the [80*13] box axis, frames in sorted order
    flat_idx = np.concatenate(
        [np.arange(t * _MAXN, t * _MAXN + int(n[t])) for t in order]
    )

    # pooling groups: runs of frames with equal box count n -> one strided
    # [128, cnt, n] max-reduce each. (frame_off, cnt, nval, col_off)
    groups = []
    col = 0
    f = 0
    for val in np.unique(ns):
        cnt = int((ns == val).sum())
        groups.append((f, cnt, int(val), col))
        f += cnt
        col += cnt * int(val)
    assert col == V and f == _BT

    return n, order, flat_idx, V, Vp, groups, _chunk_cols(Vp)


def _build(Vp, groups, chunks):
    """Build the SPMD bass program (identical on all 8 cores)."""
    import concourse.bass as bass
    import concourse.tile as tile
    from concourse import bacc, mybir

    f32 = mybir.dt.float32
    in_dt = mybir.dt.bfloat16 if _MM_DTYPE == "bf16" else f32
    cc_dt = mybir.dt.float16 if _CC_DTYPE == "f16" else f32

    nc = bacc.Bacc(
        "TRN2",
        target_bir_lowering=False,
        debug=False,
        enable_asserts=True,
        num_devices=_NC,
    )

    # packed layouts: row p, column k*width+c  <->  logical [k*128+p, c]
    x_d = nc.dram_tensor("x", [128, _KT * Vp], in_dt, kind="ExternalInput")
    w_d = nc.dram_tensor("w", [128, _KT * _NFB], in_dt, kind="ExternalInput")
    be_d = nc.dram_tensor("be", [_FC, 1], f32, kind="ExternalInput")
    wa_d = nc.dram_tensor("wa", [_FC, _A], f32, kind="ExternalInput")
    wy_d = nc.dram_tensor("wy", [_FC, _G], f32, kind="ExternalInput")
    oa_d = nc.dram_tensor("out_act", [_A, Vp], f32, kind="ExternalOutput")
    oy_d = nc.dram_tensor("out_acty", [_G, _BT], f32, kind="ExternalOutput")
    # per-chunk collective bounce buffers (internal DRAM)
    hp_d = [
        nc.dram_tensor(f"hpart{ci}", [_NFB, nsz], cc_dt)
        for ci, (_, nsz) in enumerate(chunks)
    ]
    rs_d = [
        nc.dram_tensor(f"rsout{ci}", [_FC, nsz], cc_dt)
        for ci, (_, nsz) in enumerate(chunks)
    ]

    kgroups = [(g, min(_KG, _KT - g)) for g in range(0, _KT, _KG)]

    with tile.TileContext(nc) as tc:
        with (
            tc.tile_pool(name="sb", bufs=1) as sb,
            tc.tile_pool(name="psum", bufs=1, space="PSUM") as psum,
        ):
            # load x and w in big contiguous group-DMAs; slice k-tiles in SBUF
            w_g = {}
            x_g = {}
            for g0, gn in kgroups:
                wg = sb.tile([128, gn * _NFB], in_dt, tag=f"w{g0}", bufs=1,
                             name=f"wg{g0}")
                nc.scalar.dma_start(wg[:], w_d[:, g0 * _NFB : (g0 + gn) * _NFB])
                xg = sb.tile([128, gn * Vp], in_dt, tag=f"x{g0}", bufs=1,
                             name=f"xg{g0}")
                nc.scalar.dma_start(xg[:], x_d[:, g0 * Vp : (g0 + gn) * Vp])
                for j in range(gn):
                    w_g[g0 + j] = wg[:, j * _NFB : (j + 1) * _NFB]
                    x_g[g0 + j] = xg[:, j * Vp : (j + 1) * Vp]

            # stage 1: per column chunk, H^T partial = sum_k w[k]^T @ x[k],
            # k-outer m-inner over 8 persistent PSUM banks so the PE paces
            # just above the DMA stream and stays HAM-warm.
            for ci, (co, nsz) in enumerate(chunks):
                ps = [
                    psum.tile([128, nsz], f32, tag="ps", bufs=8,
                              name=f"ps{ci}_{m}")
                    for m in range(8)
                ]
                for ki in range(_KT):
                    for m in range(8):
                        nc.tensor.matmul(
                            ps[m][:],
                            w_g[ki][:, m * 128 : (m + 1) * 128],
                            x_g[ki][:, co : co + nsz],
                            start=(ki == 0),
                            stop=(ki == _KT - 1),
                        )
                for m in range(8):
                    st = sb.tile([128, nsz], cc_dt, tag="st", bufs=6,
                                 name=f"st{ci}_{m}")
                    nc.vector.tensor_copy(st[:], ps[m][:])
                    nc.scalar.dma_start(
                        hp_d[ci][m * 128 : (m + 1) * 128, :], st[:]
                    )
                # chunk's partial is complete: reduce+scatter it across cores
                # while later chunks keep the PE busy
                nc.gpsimd.collective_compute(
                    "ReduceScatter",
                    mybir.AluOpType.add,
                    replica_groups=[list(range(_NC))],
                    ins=[hp_d[ci][:]],
                    outs=[rs_d[ci][:]],
                )

            # stage 2 (feature-sharded): bias+relu per chunk as its RS lands
            bt = sb.tile([_FC, 1], f32, tag="bt", bufs=1)
            nc.scalar.dma_start(bt[:], be_d[:])
            wa = sb.tile([_FC, _A], f32, tag="wa", bufs=1)
            nc.scalar.dma_start(wa[:], wa_d[:])
            wy = sb.tile([_FC, _G], f32, tag="wy", bufs=1)
            nc.scalar.dma_start(wy[:], wy_d[:])

            hr = sb.tile([_FC, Vp], f32, tag="hr", bufs=1)
            oa_sb = sb.tile([_A, Vp], f32, tag="oasb", bufs=1)
            for ci, (co, nsz) in enumerate(chunks):
                h2 = sb.tile([_FC, nsz], cc_dt, tag="h2", bufs=2, name=f"h2_{ci}")
                nc.scalar.dma_start(h2[:], rs_d[ci][:])
                nc.scalar.activation(
                    hr[:, co : co + nsz],
                    h2[:],
                    mybir.ActivationFunctionType.Relu,
                    bias=bt[:, 0:1],
                )
                pa = psum.tile([_A, nsz], f32, tag="ps", bufs=8, name=f"pa{ci}")
                nc.tensor.matmul(
                    pa[:], wa[:], hr[:, co : co + nsz], start=True, stop=True
                )
                nc.vector.tensor_copy(oa_sb[:, co : co + nsz], pa[:])
            nc.scalar.dma_start(oa_d[:], oa_sb[:])

            # masked max-pool: one strided reduce per group of equal box count
            pooled = sb.tile([_FC, _BT], f32, tag="pl", bufs=1)
            for fo, cnt, nv, co in groups:
                src = hr[:, co : co + cnt * nv].rearrange("p (c n) -> p c n", n=nv)
                nc.vector.reduce_max(
                    pooled[:, fo : fo + cnt], src, axis=mybir.AxisListType.X
                )

            py = psum.tile([_G, _BT], f32, tag="ps", bufs=8, name="py")
            nc.tensor.matmul(py[:], wy[:], pooled[:], start=True, stop=True)
            oy_sb = sb.tile([_G, _BT], f32, tag="oysb", bufs=1)
            nc.vector.tensor_copy(oy_sb[:], py[:])
            nc.scalar.dma_start(oy_d[:], oy_sb[:])

    nc.compile()
    return nc


def _pack_ktiles(a, width):
    """[KT*128(padded), width] -> [128, KT*width] with k-tiles side by side."""
    KT = a.shape[0] // 128
    return np.ascontiguousarray(
        a.reshape(KT, 128, width).transpose(1, 0, 2).reshape(128, KT * width)
    )


def _run(inputs, trace=False, trace_kwargs=None):
    """Shard, run on 8 cores, gather. Returns (actions, activities, results)."""
    from concourse.bass_utils import run_bass_kernel_spmd

    boxes_features_flat = np.asarray(inputs["boxes_features_flat"], np.float32)
    w_emb = np.asarray(inputs["w_emb"], np.float32)
    b_emb = np.asarray(inputs["b_emb"], np.float32)
    w_act = np.asarray(inputs["w_act"], np.float32)
    b_act = np.asarray(inputs["b_act"], np.float32)
    w_acty = np.asarray(inputs["w_acty"], np.float32)
    b_acty = np.asarray(inputs["b_acty"], np.float32)
    bboxes_num = np.asarray(inputs["bboxes_num"])

    n, order, flat_idx, V, Vp, groups, chunks = _plan(bboxes_num)

    # host marshalling: compact + transpose X to [K2D, Vp], cast, and pack
    X = boxes_features_flat.reshape(_BT * _MAXN, _K2D)
    XT = np.zeros((_K2D, Vp), np.float32)
    XT[:, :V] = X[flat_idx].T

    if _MM_DTYPE == "bf16":
        import ml_dtypes

        dt_in = ml_dtypes.bfloat16
    else:
        dt_in = np.float32
    XT = XT.astype(dt_in)
    w_in = w_emb.astype(dt_in)

    kc_pad = _KT * 128  # 3328
    in_maps = []
    for i in range(_NC):
        xi = np.zeros((kc_pad, Vp), dt_in)
        xi[:_KC] = XT[_KC * i : _KC * (i + 1)]
        wi = np.zeros((kc_pad, _NFB), dt_in)
        wi[:_KC] = w_in[_KC * i : _KC * (i + 1)]
        in_maps.append(
            {
                "x": _pack_ktiles(xi, Vp),
                "w": _pack_ktiles(wi, _NFB),
                "be": np.ascontiguousarray(
                    b_emb[_FC * i : _FC * (i + 1)].reshape(_FC, 1)
                ),
                "wa": np.ascontiguousarray(w_act[_FC * i : _FC * (i + 1)]),
                "wy": np.ascontiguousarray(w_acty[_FC * i : _FC * (i + 1)]),
            }
        )

    nc = _build(Vp, groups, chunks)
    res = run_bass_kernel_spmd(
        nc,
        in_maps,
        list(range(_NC)),
        trace=trace,
        **(trace_kwargs or {}),
    )

    # gather: sum feature-shard partials, add biases, scatter to ragged layout
    act_T = np.zeros((_A, Vp), np.float32)
    acty_T = np.zeros((_G, _BT), np.float32)
    for i in range(_NC):
        act_T += res.results[i]["out_act"]
        acty_T += res.results[i]["out_acty"]

    actions = np.zeros((_BT * _MAXN, _A), np.float32)
    actions[flat_idx] = act_T[:, :V].T + b_act[None, :]
    actions = actions.reshape(_BT, _MAXN, _A)

    activities = np.zeros((_BT, _G), np.float32)
    activities[order] = acty_T.T + b_acty[None, :]

    return actions, activities, res


def kernel(**inputs):
    actions, activities, _ = _run(inputs, trace=False)
    return actions, activities
